# revision 26
# baseline (speedup 1.0000x reference)
"""CLCNet streaming step on 8 trn2 NeuronCores (Bass/Tile).

Strategy: tensor-parallel over 8 cores.
  - stage 0/1 (magnitude scan, 322x322 fc1+bn+relu) replicated on all cores
    (W1 is tiny; replication avoids a collective).
  - GRU: row-shard W_ih/W_hh (768 gate-rows per core = 256 rows of each of
    r/z/n).  Each core produces its 256-row shard of h_new.
  - fc2+bn: column-shard W2 by the core's h_new slice -> partial y2 [2048],
    one 8KB AllReduce sums partials; relu replicated.
  - fc_out: row-shard Wout by frequency bins (21 bins/core, padded to 168);
    the final complex-multiply + sum over clc_order is done per-core with
    two constant +-1 matrices on the PE, so each core owns its bins' output.
All BN affines are folded into the weight matrices host-side; weights are
passed pre-transposed in SBUF-image layout ([128, k_chunks*M]) so every DMA
is a natural [128, N] transfer.
"""

import ml_dtypes
import numpy as np

bf16 = np.float16

import concourse.bacc as bacc
import concourse.tile as tile
import concourse.mybir as mybir
from concourse.bass_utils import run_bass_kernel_spmd

EPS = 1e-8
ALPHA = 0.99
OUT_ACT_F = 2.0
BN_EPS = 1e-5
O, F, H = 5, 161, 2048
IN = 2 * F          # 322
NCORES = 8
PCH = H // NCORES   # 256 = per-core h slice
GR = 3 * PCH        # 768 = per-core gate rows
K1 = 384            # 322 padded to 3*128
FB = 21             # freq bins per core (8*21 = 168 >= 161)
WOC = O * 2 * FB    # 210 = per-core Wout rows
DT = mybir.dt.float32
BT = mybir.dt.float16
f32 = np.float32

# column layout of the packed "smalls" [128, 222] input
_SM_LAYOUT = {}
_off = 0
for _name, _w in [("x4", 3), ("c1", 3), ("brz", 4), ("bin", 2), ("bhn", 2),
                  ("hpf", 16), ("hown", 2), ("c2", 16), ("bout", 2),
                  ("x2", 2), ("xs2", 2), ("mre", 84), ("mim", 84)]:
    _SM_LAYOUT[_name] = (_off, _w)
    _off += _w
SM_COLS = _off  # 222

# row5 packed [5, 485]: x5 [5,322] | hnr [1,161] | L4 [5,1] | a5 [1,1]
R5_COLS = 322 + 161 + 1 + 1


def _img(mat_t, kchunks, mcols):
    """[kchunks*128, mcols] -> SBUF image [128, kchunks*mcols]."""
    return np.ascontiguousarray(
        mat_t.reshape(kchunks, 128, mcols).transpose(1, 0, 2)
        .reshape(128, kchunks * mcols))


def _pf(vec, cols):
    """flat [cols*128] -> [128, cols] partition-first image."""
    return np.ascontiguousarray(vec.reshape(cols, 128).T)


def build_nc():
    nc = bacc.Bacc("TRN2", target_bir_lowering=False, debug=False,
                   num_devices=NCORES)

    i_row5 = nc.dram_tensor("row5", [5, R5_COLS], DT, kind="ExternalInput")
    i_sm = nc.dram_tensor("smalls", [128, SM_COLS], DT, kind="ExternalInput")
    i_w1 = nc.dram_tensor("w1t", [128, 3 * K1], DT, kind="ExternalInput")
    i_wih = nc.dram_tensor("wih", [128, 3 * GR], BT, kind="ExternalInput")
    i_whh = nc.dram_tensor("whh", [128, 16 * GR], BT, kind="ExternalInput")
    i_w2 = nc.dram_tensor("w2t", [128, 2 * H], BT, kind="ExternalInput")
    i_wot = nc.dram_tensor("wot", [128, 16 * WOC], BT, kind="ExternalInput")
    i_h16 = nc.dram_tensor("h16", [128, 16], BT, kind="ExternalInput")

    o_hn = nc.dram_tensor("o_hn", [1, F], DT, kind="ExternalOutput")
    o_hnew = nc.dram_tensor("o_hnew", [128, 2], DT, kind="ExternalOutput")
    o_out = nc.dram_tensor("o_out", [2 * FB, 1], DT, kind="ExternalOutput")

    AF = mybir.ActivationFunctionType

    with tile.TileContext(nc) as tc:
        with (
            tc.tile_pool(name="sb", bufs=1) as sb,
            tc.tile_pool(name="ps", bufs=1, space="PSUM") as ps,
            tc.tile_pool(name="dram", bufs=1, space="DRAM") as dram,
        ):
            # ---- input DMAs (emission order ~ priority) ----
            row5 = sb.tile([5, R5_COLS], DT)
            nc.sync.dma_start(row5[:], i_row5[:])
            sm = sb.tile([128, SM_COLS], DT)
            nc.sync.dma_start(sm[:], i_sm[:])
            w1t = sb.tile([128, 3 * K1], DT)
            nc.sync.dma_start(w1t[:], i_w1[:])
            wih = sb.tile([128, 3 * GR], BT)
            nc.sync.dma_start(wih[:], i_wih[:])
            whh = []
            for s in range(4):  # 4 slabs x 4 k-chunks
                t = sb.tile([128, 4 * GR], BT, tag=f"whh{s}")
                nc.sync.dma_start(t[:], i_whh[:, s * 4 * GR:(s + 1) * 4 * GR])
                whh.append(t)
            w2 = []
            for s in range(2):
                t = sb.tile([128, H], BT, tag=f"w2{s}")
                nc.sync.dma_start(t[:], i_w2[:, s * H:(s + 1) * H])
                w2.append(t)
            wot = []
            for s in range(2):
                t = sb.tile([128, 8 * WOC], BT, tag=f"wot{s}")
                nc.sync.dma_start(t[:], i_wot[:, s * 8 * WOC:(s + 1) * 8 * WOC])
                wot.append(t)

            hpf16 = sb.tile([128, 16], BT)
            nc.sync.dma_start(hpf16[:], i_h16[:])

            def smv(name):
                o, w = _SM_LAYOUT[name]
                return sm[:, o:o + w]

            x5 = row5[:, 0:IN]                 # [5,322]
            hnr = row5[0:1, IN:IN + F]         # [1,161]
            L4v = row5[:, IN + F:IN + F + 1]   # [5,1]
            a5v = row5[0:1, IN + F + 1:IN + F + 2]  # [1,1]

            zb = sb.tile([128, 1], DT)         # zero bias for activations
            nc.vector.memset(zb[:], 0.0)
            eps5 = sb.tile([5, 1], DT)
            nc.vector.memset(eps5[:], EPS)

            # ---- stage 0: magnitude + exp-decay scan (as 5x5 matmul) ----
            sq = sb.tile([5, IN], DT)
            nc.vector.tensor_mul(sq[:], x5, x5)
            sqv = sq[:].rearrange("p (f two) -> p f two", two=2)
            m2 = sb.tile([5, F], DT)
            nc.vector.tensor_add(m2[:], sqv[:, :, 0], sqv[:, :, 1])
            xm = sb.tile([5, F], DT)
            nc.scalar.activation(xm[:], m2[:], AF.Sqrt, bias=eps5[:])

            S4 = ps.tile([1, F], DT, tag="b0")
            nc.tensor.matmul(S4[:], L4v, xm[:], start=True, stop=False)
            nc.tensor.matmul(S4[:], a5v, hnr, start=False, stop=True)

            s4_sb = sb.tile([1, F], DT)
            nc.vector.tensor_copy(s4_sb[:], S4[:])
            nc.sync.dma_start(o_hn[:], s4_sb[:])

            den = sb.tile([1, F], DT)
            nc.vector.tensor_scalar_add(den[:], s4_sb[:], EPS)
            rec = sb.tile([1, F], DT)
            nc.vector.reciprocal(rec[:], den[:])
            rexp = sb.tile([1, IN], DT)
            rexpv = rexp[:].rearrange("p (f two) -> p f two", two=2)
            nc.vector.tensor_copy(rexpv[:, :, 0], rec[:])
            nc.vector.tensor_copy(rexpv[:, :, 1], rec[:])

            ones11 = sb.tile([1, 1], DT)
            nc.vector.memset(ones11[:], 1.0)

            # transpose rexp (free layout) into partition layout via PE
            vt = ps.tile([128, 3], DT, tag="b1")
            nc.tensor.matmul(vt[:, 0:1], rexp[0:1, 0:128], ones11[:],
                             start=True, stop=True)
            nc.tensor.matmul(vt[:, 1:2], rexp[0:1, 128:256], ones11[:],
                             start=True, stop=True)
            nc.tensor.matmul(vt[0:66, 2:3], rexp[0:1, 256:322], ones11[:],
                             start=True, stop=True)

            v_pf = sb.tile([128, 3], DT)
            nc.vector.memset(v_pf[:], 0.0)
            nc.vector.tensor_mul(v_pf[:, 0:2], vt[:, 0:2], smv("x4")[:, 0:2])
            nc.vector.tensor_mul(v_pf[0:66, 2:3], vt[0:66, 2:3],
                                 smv("x4")[0:66, 2:3])

            # ---- stage 1: fc1+bn+relu (replicated) ----
            # m-chunk outer: sequential accumulation groups share one bank
            y1p = ps.tile([128, 3], DT, tag="b2")
            for i in range(3):
                for j in range(3):
                    nc.tensor.matmul(
                        y1p[:, i:i + 1],
                        w1t[:, K1 * j + 128 * i: K1 * j + 128 * (i + 1)],
                        v_pf[:, j:j + 1], start=(j == 0), stop=(j == 2))
            y1 = sb.tile([128, 3], BT)
            for i in range(3):
                nc.scalar.activation(y1[:, i:i + 1], y1p[:, i:i + 1], AF.Relu,
                                     bias=smv("c1")[:, i:i + 1])

            # ---- stage 2: GRU (row-sharded) ----
            # one PSUM bank per concurrent accumulation group; groups start
            # with the streaming W_hh part so PE can run before y1 is ready
            grz = [ps.tile([128, 1], DT, tag=f"b{3 + i}", name=f"grz{i}")
                   for i in range(4)]
            gin = ps.tile([128, 2], DT, tag="b7")    # n gate, ih part
            ghn = [ps.tile([128, 1], DT, tag=t, name=f"ghn{t}")
                   for t in ("b0", "b1")]
            for j2 in range(16):    # k-chunks of h
                s, jj = divmod(j2, 4)
                for mi in range(4):
                    nc.tensor.matmul(
                        grz[mi][:],
                        whh[s][:, GR * jj + 128 * mi: GR * jj + 128 * (mi + 1)],
                        hpf16[:, j2:j2 + 1], start=(j2 == 0), stop=False)
                for m2_ in range(2):
                    mi = 4 + m2_
                    nc.tensor.matmul(
                        ghn[m2_][:],
                        whh[s][:, GR * jj + 128 * mi: GR * jj + 128 * (mi + 1)],
                        hpf16[:, j2:j2 + 1], start=(j2 == 0), stop=(j2 == 15))
            for j in range(3):      # k-chunks of y1 (finish the r/z groups)
                for mi in range(4):
                    nc.tensor.matmul(
                        grz[mi][:],
                        wih[:, GR * j + 128 * mi: GR * j + 128 * (mi + 1)],
                        y1[:, j:j + 1], start=False, stop=(j == 2))
            for m2_ in range(2):    # n gate ih part: sequential groups
                mi = 4 + m2_
                for j in range(3):
                    nc.tensor.matmul(
                        gin[:, m2_:m2_ + 1],
                        wih[:, GR * j + 128 * mi: GR * j + 128 * (mi + 1)],
                        y1[:, j:j + 1], start=(j == 0), stop=(j == 2))

            r_sb = sb.tile([128, 2], DT)
            z_sb = sb.tile([128, 2], DT)
            for i in range(2):
                nc.scalar.activation(r_sb[:, i:i + 1], grz[i][:],
                                     AF.Sigmoid, bias=smv("brz")[:, i:i + 1])
                nc.scalar.activation(z_sb[:, i:i + 1], grz[2 + i][:],
                                     AF.Sigmoid, bias=smv("brz")[:, 2 + i:3 + i])
            t1 = sb.tile([128, 2], DT)
            t2 = sb.tile([128, 2], DT)
            for i in range(2):
                nc.vector.tensor_scalar_add(t1[:, i:i + 1], gin[:, i:i + 1],
                                            smv("bin")[:, i:i + 1])
                nc.vector.tensor_scalar_add(t2[:, i:i + 1], ghn[i][:],
                                            smv("bhn")[:, i:i + 1])
            t3 = sb.tile([128, 2], DT)
            nc.vector.tensor_mul(t3[:], r_sb[:], t2[:])
            t4 = sb.tile([128, 2], DT)
            nc.vector.tensor_add(t4[:], t1[:], t3[:])
            n_sb = sb.tile([128, 2], DT)
            nc.scalar.activation(n_sb[:], t4[:], AF.Tanh, bias=zb[:])
            t5 = sb.tile([128, 2], DT)
            nc.vector.tensor_sub(t5[:], smv("hown"), n_sb[:])
            t6 = sb.tile([128, 2], DT)
            nc.vector.tensor_mul(t6[:], z_sb[:], t5[:])
            hn_sb = sb.tile([128, 2], DT)
            nc.vector.tensor_add(hn_sb[:], n_sb[:], t6[:])
            nc.sync.dma_start(o_hnew[:], hn_sb[:])
            hn16 = sb.tile([128, 2], BT)
            nc.vector.tensor_copy(hn16[:], hn_sb[:])

            # ---- stage 3: fc2 partial (column-sharded) + AllReduce ----
            # m-chunk outer: 16 sequential groups in one bank
            y2p = ps.tile([128, 16], DT, tag="b2")
            for mi in range(16):
                for j in range(2):
                    nc.tensor.matmul(
                        y2p[:, mi:mi + 1],
                        w2[j][:, 128 * mi:128 * (mi + 1)],
                        hn16[:, j:j + 1], start=(j == 0), stop=(j == 1))
            y2sb = sb.tile([128, 16], BT)
            nc.vector.tensor_add(y2sb[:], y2p[:], smv("c2"))

            # AllGather the 8 partials (copy-only, ~2x faster than ncfw
            # AllReduce at this size), then tree-sum locally in fp32 on DVE
            cc_in = dram.tile([128, 16], BT)
            cc_out = dram.tile([NCORES * 128, 16], BT, addr_space="Shared")
            nc.sync.dma_start(cc_in[:], y2sb[:])
            nc.gpsimd.collective_compute(
                "AllGather", mybir.AluOpType.bypass,
                replica_groups=[list(range(NCORES))],
                ins=[cc_in[:].opt()], outs=[cc_out[:].opt()])
            y2all = sb.tile([128, NCORES * 16], BT)
            nc.sync.dma_start(
                y2all[:].rearrange("p (r f) -> p r f", r=NCORES),
                cc_out[:].rearrange("(r p) f -> p r f", p=128))
            ts1 = sb.tile([128, 64], DT)
            nc.vector.tensor_add(ts1[:], y2all[:, 0:64], y2all[:, 64:128])
            ts2 = sb.tile([128, 32], DT)
            nc.vector.tensor_add(ts2[:], ts1[:, 0:32], ts1[:, 32:64])
            y2f = sb.tile([128, 16], DT)
            nc.vector.tensor_add(y2f[:], ts2[:, 0:16], ts2[:, 16:32])

            u_sb = sb.tile([128, 16], BT)
            nc.scalar.activation(u_sb[:], y2f[:], AF.Relu, bias=zb[:])

            # ---- stage 4: fc_out (bin-sharded) + tanh ----
            coefp = ps.tile([128, 2], DT, tag="b3")
            for j in range(16):
                s, jj = divmod(j, 8)
                base = WOC * jj
                nc.tensor.matmul(coefp[:, 0:1],
                                 wot[s][:, base:base + 128],
                                 u_sb[:, j:j + 1], start=(j == 0), stop=(j == 15))
            for j in range(16):
                s, jj = divmod(j, 8)
                base = WOC * jj
                nc.tensor.matmul(coefp[0:82, 1:2],
                                 wot[s][:, base + 128:base + WOC],
                                 u_sb[:, j:j + 1], start=(j == 0), stop=(j == 15))
            coef = sb.tile([128, 2], DT)
            nc.vector.memset(coef[:], 0.0)
            nc.scalar.activation(coef[:, 0:1], coefp[:, 0:1], AF.Tanh,
                                 bias=smv("bout")[:, 0:1])
            nc.scalar.activation(coef[0:82, 1:2], coefp[0:82, 1:2], AF.Tanh,
                                 bias=smv("bout")[0:82, 1:2])

            # ---- final: complex multiply + sum over clc_order ----
            p1 = sb.tile([128, 2], DT)
            nc.vector.tensor_mul(p1[:], smv("x2"), coef[:])
            p2 = sb.tile([128, 2], DT)
            nc.vector.tensor_mul(p2[:], smv("xs2"), coef[:])

            osum = ps.tile([2 * FB, 1], DT, tag="b4")
            mre = smv("mre")
            mim = smv("mim")
            for j in range(2):
                nc.tensor.matmul(osum[:], mre[:, 42 * j:42 * (j + 1)],
                                 p1[:, j:j + 1], start=(j == 0), stop=False)
            for j in range(2):
                nc.tensor.matmul(osum[:], mim[:, 42 * j:42 * (j + 1)],
                                 p2[:, j:j + 1], start=False, stop=(j == 1))
            ot = sb.tile([2 * FB, 1], DT)
            nc.scalar.activation(ot[:], osum[:], AF.Copy)
            nc.sync.dma_start(o_out[:], ot[:])

    nc.compile()
    return nc


def prep_inputs(x, h_norm, h_rnn, W1, b1, g1, beta1, rm1, rv1,
                W_ih, W_hh, b_ih, b_hh, W2, b2, g2, beta2, rm2, rv2,
                Wout, bout):
    """Host-side prep: fold BN, transpose/pad/shard weights, pack smalls."""
    x = np.asarray(x, f32)
    h = np.asarray(h_rnn, f32).reshape(H)
    h_norm = np.asarray(h_norm, f32)

    # row5 (same on all cores)
    L4 = np.array([(1.0 - ALPHA) * ALPHA ** (4 - t) for t in range(5)], f32)
    row5 = np.zeros((5, R5_COLS), f32)
    row5[:, :IN] = x.reshape(5, IN)
    row5[0, IN:IN + F] = h_norm
    row5[:, IN + F] = L4
    row5[0, IN + F + 1] = ALPHA ** 5

    # fc1 + bn1 folded
    s1 = (np.asarray(g1, f32) / np.sqrt(np.asarray(rv1, f32) + BN_EPS))
    W1s = np.asarray(W1, f32) * s1[:, None]
    c1 = (np.asarray(b1, f32) - np.asarray(rm1, f32)) * s1 + np.asarray(beta1, f32)
    W1sT = np.zeros((K1, K1), f32)
    W1sT[:IN, :IN] = W1s.T
    w1t_img = _img(W1sT, 3, K1)
    c1_pf = _pf(np.pad(c1, (0, K1 - IN)), 3)

    x4d50 = np.pad(x[O - 1].reshape(IN) / 50.0, (0, K1 - IN)).astype(f32)
    x4_pf = _pf(x4d50, 3)

    # fc2 + bn2 folded
    s2 = (np.asarray(g2, f32) / np.sqrt(np.asarray(rv2, f32) + BN_EPS))
    W2s = np.asarray(W2, f32) * s2[:, None]
    c2 = (np.asarray(b2, f32) - np.asarray(rm2, f32)) * s2 + np.asarray(beta2, f32)
    c2_pf = _pf(c2 / NCORES, 16)
    hpf = _pf(h, 16)

    bsum = np.asarray(b_ih, f32) + np.asarray(b_hh, f32)
    W_ih = np.asarray(W_ih, f32)
    W_hh = np.asarray(W_hh, f32)
    Wout = np.asarray(Wout, f32)
    bout = np.asarray(bout, f32)
    wob = Wout.reshape(O, F, 2, H)
    bob = bout.reshape(O, F, 2)

    # +-1 reduction matrices for the final complex-mul (same on all cores)
    Mre = np.zeros((256, 2 * FB), f32)
    Mim = np.zeros((256, 2 * FB), f32)
    for t in range(WOC):
        o_, gci = divmod(t, 2 * FB)
        g, ci = divmod(gci, 2)
        Mre[t, 2 * g] = 1.0 if ci == 0 else -1.0
        Mim[t, 2 * g + 1] = 1.0
    mre_img = _img(Mre, 2, 2 * FB)
    mim_img = _img(Mim, 2, 2 * FB)

    in_maps = []
    for c in range(NCORES):
        gr = np.arange(PCH * c, PCH * (c + 1))
        idx = np.concatenate([gr, H + gr, 2 * H + gr])
        WihT = np.zeros((K1, GR), f32)
        WihT[:IN, :] = W_ih[idx, :].T
        wih_img = _img(WihT, 3, GR).astype(bf16)
        whh_img = _img(np.ascontiguousarray(W_hh[idx, :].T), 16, GR).astype(bf16)

        brz_pf = _pf(np.concatenate([bsum[gr], bsum[H + gr]]), 4)
        bin_pf = _pf(np.asarray(b_ih, f32)[2 * H + gr], 2)
        bhn_pf = _pf(np.asarray(b_hh, f32)[2 * H + gr], 2)
        hown_pf = _pf(h[gr], 2)

        w2_img = _img(np.ascontiguousarray(W2s[:, gr].T), 2, H).astype(bf16)

        # Wout rows for this core's bins, o-major
        WoT = np.zeros((H, WOC), f32)
        bo_c = np.zeros(WOC, f32)
        x2v = np.zeros(256, f32)
        xs2v = np.zeros(256, f32)
        f0 = FB * c
        for o_ in range(O):
            for g in range(FB):
                f = f0 + g
                if f >= F:
                    continue
                for ci in range(2):
                    t = o_ * 2 * FB + 2 * g + ci
                    WoT[:, t] = wob[o_, f, ci, :]
                    bo_c[t] = bob[o_, f, ci]
                    x2v[t] = 2.0 * x[o_, f, ci]
                    xs2v[t] = 2.0 * x[o_, f, 1 - ci]
        wot_img = _img(WoT, 16, WOC).astype(bf16)
        bout_pf = _pf(np.pad(bo_c, (0, 256 - WOC)), 2)

        smalls = np.zeros((128, SM_COLS), f32)

        def put(name, arr):
            o_, w = _SM_LAYOUT[name]
            smalls[:, o_:o_ + w] = arr

        put("x4", x4_pf)
        put("c1", c1_pf)
        put("brz", brz_pf)
        put("bin", bin_pf)
        put("bhn", bhn_pf)
        put("hpf", hpf)
        put("hown", hown_pf)
        put("c2", c2_pf)
        put("bout", bout_pf)
        put("x2", _pf(x2v, 2))
        put("xs2", _pf(xs2v, 2))
        put("mre", mre_img)
        put("mim", mim_img)

        in_maps.append({
            "row5": row5, "smalls": smalls, "w1t": w1t_img,
            "wih": wih_img, "whh": whh_img, "w2t": w2_img, "wot": wot_img,
            "h16": hpf.astype(bf16),
        })
    return in_maps


def assemble(results):
    out_full = np.concatenate(
        [results[c]["o_out"].ravel() for c in range(NCORES)])[:IN]
    out = out_full.reshape(F, 2).astype(f32)
    h_norm_new = results[0]["o_hn"].reshape(F).astype(f32)
    h_new = np.concatenate(
        [results[c]["o_hnew"].T.ravel() for c in range(NCORES)])
    return out, h_norm_new, h_new.reshape(1, 1, H).astype(f32)


_NC_CACHE = [None]


def run(inputs, trace=False, tmpdir=None):
    if _NC_CACHE[0] is None:
        _NC_CACHE[0] = build_nc()
    nc = _NC_CACHE[0]
    in_maps = prep_inputs(**inputs)
    res = run_bass_kernel_spmd(nc, in_maps, core_ids=list(range(NCORES)),
                               trace=trace, tmpdir=tmpdir)
    return assemble(res.results), res


def kernel(**inputs):
    (out, h_norm_new, h_new), _ = run(inputs)
    return out, h_norm_new, h_new


# revision 27
# speedup vs baseline: 1.1110x; 1.1110x over previous
"""CLCNet streaming step on 8 trn2 NeuronCores (Bass/Tile).

Strategy: tensor-parallel over 8 cores.
  - stage 0/1 (magnitude scan, 322x322 fc1+bn+relu) replicated on all cores
    (W1 is tiny; replication avoids a collective).
  - GRU: row-shard W_ih/W_hh (768 gate-rows per core = 256 rows of each of
    r/z/n).  Each core produces its 256-row shard of h_new.
  - fc2+bn: column-shard W2 by the core's h_new slice -> partial y2 [2048],
    one 8KB AllReduce sums partials; relu replicated.
  - fc_out: row-shard Wout by frequency bins (21 bins/core, padded to 168);
    the final complex-multiply + sum over clc_order is done per-core with
    two constant +-1 matrices on the PE, so each core owns its bins' output.
All BN affines are folded into the weight matrices host-side; weights are
passed pre-transposed in SBUF-image layout ([128, k_chunks*M]) so every DMA
is a natural [128, N] transfer.
"""

import ml_dtypes
import numpy as np

bf16 = np.float16

import concourse.bacc as bacc
import concourse.tile as tile
import concourse.mybir as mybir
from concourse.bass_utils import run_bass_kernel_spmd

EPS = 1e-8
ALPHA = 0.99
OUT_ACT_F = 2.0
BN_EPS = 1e-5
O, F, H = 5, 161, 2048
IN = 2 * F          # 322
NCORES = 8
PCH = H // NCORES   # 256 = per-core h slice
GR = 3 * PCH        # 768 = per-core gate rows
K1 = 384            # 322 padded to 3*128
FB = 21             # freq bins per core (8*21 = 168 >= 161)
WOC = O * 2 * FB    # 210 = per-core Wout rows
DT = mybir.dt.float32
BT = mybir.dt.float16
f32 = np.float32

# column layout of the packed "smalls" [128, 222] input
_SM_LAYOUT = {}
_off = 0
for _name, _w in [("x4", 3), ("c1", 3), ("brz", 4), ("bin", 2), ("bhn", 2),
                  ("hpf", 16), ("hown", 2), ("c2", 16), ("bout", 2),
                  ("x2", 2), ("xs2", 2), ("mre", 84), ("mim", 84)]:
    _SM_LAYOUT[_name] = (_off, _w)
    _off += _w
SM_COLS = _off  # 222

# row5 packed [5, 485]: x5 [5,322] | hnr [1,161] | L4 [5,1] | a5 [1,1]
R5_COLS = 322 + 161 + 1 + 1


def _img(mat_t, kchunks, mcols):
    """[kchunks*128, mcols] -> SBUF image [128, kchunks*mcols]."""
    return np.ascontiguousarray(
        mat_t.reshape(kchunks, 128, mcols).transpose(1, 0, 2)
        .reshape(128, kchunks * mcols))


def _pf(vec, cols):
    """flat [cols*128] -> [128, cols] partition-first image."""
    return np.ascontiguousarray(vec.reshape(cols, 128).T)


def build_nc():
    nc = bacc.Bacc("TRN2", target_bir_lowering=False, debug=False,
                   num_devices=NCORES)

    i_row5 = nc.dram_tensor("row5", [5, R5_COLS], DT, kind="ExternalInput")
    i_sm = nc.dram_tensor("smalls", [128, SM_COLS], DT, kind="ExternalInput")
    i_w1 = nc.dram_tensor("w1t", [128, 3 * K1], DT, kind="ExternalInput")
    i_wih = nc.dram_tensor("wih", [128, 3 * GR], BT, kind="ExternalInput")
    i_whh = nc.dram_tensor("whh", [128, 16 * GR], BT, kind="ExternalInput")
    i_w2 = nc.dram_tensor("w2t", [128, 2 * H], BT, kind="ExternalInput")
    i_wot = nc.dram_tensor("wot", [128, 16 * WOC], BT, kind="ExternalInput")
    i_h16 = nc.dram_tensor("h16", [128, 16], BT, kind="ExternalInput")

    o_hn = nc.dram_tensor("o_hn", [1, F], DT, kind="ExternalOutput")
    o_hnew = nc.dram_tensor("o_hnew", [128, 2], DT, kind="ExternalOutput")
    o_out = nc.dram_tensor("o_out", [2 * FB, 1], DT, kind="ExternalOutput")

    AF = mybir.ActivationFunctionType

    with tile.TileContext(nc) as tc:
        with (
            tc.tile_pool(name="sb", bufs=1) as sb,
            tc.tile_pool(name="ps", bufs=1, space="PSUM") as ps,
            tc.tile_pool(name="dram", bufs=1, space="DRAM") as dram,
        ):
            # ---- input DMAs (emission order ~ priority) ----
            row5 = sb.tile([5, R5_COLS], DT)
            nc.sync.dma_start(row5[:], i_row5[:])
            sm = sb.tile([128, SM_COLS], DT)
            nc.sync.dma_start(sm[:], i_sm[:])
            w1t = sb.tile([128, 3 * K1], DT)
            nc.sync.dma_start(w1t[:], i_w1[:])
            wih = sb.tile([128, 3 * GR], BT)
            nc.sync.dma_start(wih[:], i_wih[:])
            whh = []
            for s in range(4):  # 4 slabs x 4 k-chunks
                t = sb.tile([128, 4 * GR], BT, tag=f"whh{s}")
                nc.sync.dma_start(t[:], i_whh[:, s * 4 * GR:(s + 1) * 4 * GR])
                whh.append(t)
            w2 = []
            for s in range(2):
                t = sb.tile([128, H], BT, tag=f"w2{s}")
                nc.sync.dma_start(t[:], i_w2[:, s * H:(s + 1) * H])
                w2.append(t)
            wot = []
            for s in range(2):
                t = sb.tile([128, 8 * WOC], BT, tag=f"wot{s}")
                nc.sync.dma_start(t[:], i_wot[:, s * 8 * WOC:(s + 1) * 8 * WOC])
                wot.append(t)

            hpf16 = sb.tile([128, 16], BT)
            nc.sync.dma_start(hpf16[:], i_h16[:])

            def smv(name):
                o, w = _SM_LAYOUT[name]
                return sm[:, o:o + w]

            x5 = row5[:, 0:IN]                 # [5,322]
            hnr = row5[0:1, IN:IN + F]         # [1,161]
            L4v = row5[:, IN + F:IN + F + 1]   # [5,1]
            a5v = row5[0:1, IN + F + 1:IN + F + 2]  # [1,1]

            zb = sb.tile([128, 1], DT)         # zero bias for activations
            nc.vector.memset(zb[:], 0.0)
            eps5 = sb.tile([5, 1], DT)
            nc.vector.memset(eps5[:], EPS)

            # ---- stage 0: magnitude + exp-decay scan (as 5x5 matmul) ----
            sq = sb.tile([5, IN], DT)
            nc.vector.tensor_mul(sq[:], x5, x5)
            sqv = sq[:].rearrange("p (f two) -> p f two", two=2)
            m2 = sb.tile([5, F], DT)
            nc.vector.tensor_add(m2[:], sqv[:, :, 0], sqv[:, :, 1])
            xm = sb.tile([5, F], DT)
            nc.scalar.activation(xm[:], m2[:], AF.Sqrt, bias=eps5[:])

            S4 = ps.tile([1, F], DT, tag="b0")
            nc.tensor.matmul(S4[:], L4v, xm[:], start=True, stop=False)
            nc.tensor.matmul(S4[:], a5v, hnr, start=False, stop=True)

            s4_sb = sb.tile([1, F], DT)
            nc.vector.tensor_copy(s4_sb[:], S4[:])
            nc.sync.dma_start(o_hn[:], s4_sb[:])

            den = sb.tile([1, F], DT)
            nc.vector.tensor_scalar_add(den[:], s4_sb[:], EPS)
            rec = sb.tile([1, F], DT)
            nc.vector.reciprocal(rec[:], den[:])
            rexp = sb.tile([1, IN], DT)
            rexpv = rexp[:].rearrange("p (f two) -> p f two", two=2)
            nc.vector.tensor_copy(rexpv[:, :, 0], rec[:])
            nc.vector.tensor_copy(rexpv[:, :, 1], rec[:])

            ones11 = sb.tile([1, 1], DT)
            nc.vector.memset(ones11[:], 1.0)

            # transpose rexp (free layout) into partition layout via PE
            vt = ps.tile([128, 3], DT, tag="b1")
            nc.tensor.matmul(vt[:, 0:1], rexp[0:1, 0:128], ones11[:],
                             start=True, stop=True)
            nc.tensor.matmul(vt[:, 1:2], rexp[0:1, 128:256], ones11[:],
                             start=True, stop=True)
            nc.tensor.matmul(vt[0:66, 2:3], rexp[0:1, 256:322], ones11[:],
                             start=True, stop=True)

            v_pf = sb.tile([128, 3], DT)
            nc.vector.memset(v_pf[:], 0.0)
            nc.vector.tensor_mul(v_pf[:, 0:2], vt[:, 0:2], smv("x4")[:, 0:2])
            nc.vector.tensor_mul(v_pf[0:66, 2:3], vt[0:66, 2:3],
                                 smv("x4")[0:66, 2:3])

            # ---- stage 1: fc1+bn+relu (replicated) ----
            # m-chunk outer: sequential accumulation groups share one bank
            y1p = ps.tile([128, 3], DT, tag="b2")
            for i in range(3):
                for j in range(3):
                    nc.tensor.matmul(
                        y1p[:, i:i + 1],
                        w1t[:, K1 * j + 128 * i: K1 * j + 128 * (i + 1)],
                        v_pf[:, j:j + 1], start=(j == 0), stop=(j == 2))
            y1 = sb.tile([128, 3], BT)
            for i in range(3):
                nc.scalar.activation(y1[:, i:i + 1], y1p[:, i:i + 1], AF.Relu,
                                     bias=smv("c1")[:, i:i + 1])

            # ---- stage 2: GRU (row-sharded) ----
            # one PSUM bank per concurrent accumulation group; groups start
            # with the streaming W_hh part so PE can run before y1 is ready
            grz = [ps.tile([128, 1], DT, tag=f"b{3 + i}", name=f"grz{i}")
                   for i in range(4)]
            gin = ps.tile([128, 2], DT, tag="b7")    # n gate, ih part
            ghn = [ps.tile([128, 1], DT, tag=t, name=f"ghn{t}")
                   for t in ("b0", "b1")]
            for j2 in range(16):    # k-chunks of h
                s, jj = divmod(j2, 4)
                for mi in range(4):
                    nc.tensor.matmul(
                        grz[mi][:],
                        whh[s][:, GR * jj + 128 * mi: GR * jj + 128 * (mi + 1)],
                        hpf16[:, j2:j2 + 1], start=(j2 == 0), stop=False)
                for m2_ in range(2):
                    mi = 4 + m2_
                    nc.tensor.matmul(
                        ghn[m2_][:],
                        whh[s][:, GR * jj + 128 * mi: GR * jj + 128 * (mi + 1)],
                        hpf16[:, j2:j2 + 1], start=(j2 == 0), stop=(j2 == 15))
            for j in range(3):      # k-chunks of y1 (finish the r/z groups)
                for mi in range(4):
                    nc.tensor.matmul(
                        grz[mi][:],
                        wih[:, GR * j + 128 * mi: GR * j + 128 * (mi + 1)],
                        y1[:, j:j + 1], start=False, stop=(j == 2))
            for m2_ in range(2):    # n gate ih part: sequential groups
                mi = 4 + m2_
                for j in range(3):
                    nc.tensor.matmul(
                        gin[:, m2_:m2_ + 1],
                        wih[:, GR * j + 128 * mi: GR * j + 128 * (mi + 1)],
                        y1[:, j:j + 1], start=(j == 0), stop=(j == 2))

            r_sb = sb.tile([128, 2], DT)
            z_sb = sb.tile([128, 2], DT)
            for i in range(2):
                nc.scalar.activation(r_sb[:, i:i + 1], grz[i][:],
                                     AF.Sigmoid, bias=smv("brz")[:, i:i + 1])
                nc.scalar.activation(z_sb[:, i:i + 1], grz[2 + i][:],
                                     AF.Sigmoid, bias=smv("brz")[:, 2 + i:3 + i])
            t1 = sb.tile([128, 2], DT)
            t2 = sb.tile([128, 2], DT)
            for i in range(2):
                nc.vector.tensor_scalar_add(t1[:, i:i + 1], gin[:, i:i + 1],
                                            smv("bin")[:, i:i + 1])
                nc.vector.tensor_scalar_add(t2[:, i:i + 1], ghn[i][:],
                                            smv("bhn")[:, i:i + 1])
            t3 = sb.tile([128, 2], DT)
            nc.vector.tensor_mul(t3[:], r_sb[:], t2[:])
            t4 = sb.tile([128, 2], DT)
            nc.vector.tensor_add(t4[:], t1[:], t3[:])
            n_sb = sb.tile([128, 2], DT)
            nc.scalar.activation(n_sb[:], t4[:], AF.Tanh, bias=zb[:])
            t5 = sb.tile([128, 2], DT)
            nc.vector.tensor_sub(t5[:], smv("hown"), n_sb[:])
            t6 = sb.tile([128, 2], DT)
            nc.vector.tensor_mul(t6[:], z_sb[:], t5[:])
            hn_sb = sb.tile([128, 2], DT)
            nc.vector.tensor_add(hn_sb[:], n_sb[:], t6[:])
            nc.sync.dma_start(o_hnew[:], hn_sb[:])
            hn16 = sb.tile([128, 2], BT)
            nc.vector.tensor_copy(hn16[:], hn_sb[:])

            # ---- stage 3: fc2 partial (column-sharded) + AllReduce ----
            # m-chunk outer: 16 sequential groups in one bank
            y2p = ps.tile([128, 16], DT, tag="b2")
            for mi in range(16):
                for j in range(2):
                    nc.tensor.matmul(
                        y2p[:, mi:mi + 1],
                        w2[j][:, 128 * mi:128 * (mi + 1)],
                        hn16[:, j:j + 1], start=(j == 0), stop=(j == 1))
            y2sb = sb.tile([128, 16], DT)
            nc.vector.tensor_add(y2sb[:], y2p[:], smv("c2"))

            # AllGather the 8 partials (copy-only, ~2x faster than ncfw
            # AllReduce at this size), then tree-sum locally in fp32 on DVE
            cc_in = dram.tile([128, 16], DT)
            cc_out = dram.tile([NCORES * 128, 16], DT, addr_space="Shared")
            nc.sync.dma_start(cc_in[:], y2sb[:])
            nc.gpsimd.collective_compute(
                "AllGather", mybir.AluOpType.bypass,
                replica_groups=[list(range(NCORES))],
                ins=[cc_in[:].opt()], outs=[cc_out[:].opt()])
            y2all = sb.tile([128, NCORES * 16], DT)
            nc.sync.dma_start(
                y2all[:].rearrange("p (r f) -> p r f", r=NCORES),
                cc_out[:].rearrange("(r p) f -> p r f", p=128))
            ts1 = sb.tile([128, 64], DT)
            nc.vector.tensor_add(ts1[:], y2all[:, 0:64], y2all[:, 64:128])
            ts2 = sb.tile([128, 32], DT)
            nc.vector.tensor_add(ts2[:], ts1[:, 0:32], ts1[:, 32:64])
            y2f = sb.tile([128, 16], DT)
            nc.vector.tensor_add(y2f[:], ts2[:, 0:16], ts2[:, 16:32])

            u_sb = sb.tile([128, 16], BT)
            nc.scalar.activation(u_sb[:], y2f[:], AF.Relu, bias=zb[:])

            # ---- stage 4: fc_out (bin-sharded) + tanh ----
            coefp = ps.tile([128, 2], DT, tag="b3")
            for j in range(16):
                s, jj = divmod(j, 8)
                base = WOC * jj
                nc.tensor.matmul(coefp[:, 0:1],
                                 wot[s][:, base:base + 128],
                                 u_sb[:, j:j + 1], start=(j == 0), stop=(j == 15))
            for j in range(16):
                s, jj = divmod(j, 8)
                base = WOC * jj
                nc.tensor.matmul(coefp[0:82, 1:2],
                                 wot[s][:, base + 128:base + WOC],
                                 u_sb[:, j:j + 1], start=(j == 0), stop=(j == 15))
            coef = sb.tile([128, 2], DT)
            nc.vector.memset(coef[:], 0.0)
            nc.scalar.activation(coef[:, 0:1], coefp[:, 0:1], AF.Tanh,
                                 bias=smv("bout")[:, 0:1])
            nc.scalar.activation(coef[0:82, 1:2], coefp[0:82, 1:2], AF.Tanh,
                                 bias=smv("bout")[0:82, 1:2])

            # ---- final: complex multiply + sum over clc_order ----
            p1 = sb.tile([128, 2], DT)
            nc.vector.tensor_mul(p1[:], smv("x2"), coef[:])
            p2 = sb.tile([128, 2], DT)
            nc.vector.tensor_mul(p2[:], smv("xs2"), coef[:])

            osum = ps.tile([2 * FB, 1], DT, tag="b4")
            mre = smv("mre")
            mim = smv("mim")
            for j in range(2):
                nc.tensor.matmul(osum[:], mre[:, 42 * j:42 * (j + 1)],
                                 p1[:, j:j + 1], start=(j == 0), stop=False)
            for j in range(2):
                nc.tensor.matmul(osum[:], mim[:, 42 * j:42 * (j + 1)],
                                 p2[:, j:j + 1], start=False, stop=(j == 1))
            ot = sb.tile([2 * FB, 1], DT)
            nc.scalar.activation(ot[:], osum[:], AF.Copy)
            nc.sync.dma_start(o_out[:], ot[:])

    nc.compile()
    return nc


def prep_inputs(x, h_norm, h_rnn, W1, b1, g1, beta1, rm1, rv1,
                W_ih, W_hh, b_ih, b_hh, W2, b2, g2, beta2, rm2, rv2,
                Wout, bout):
    """Host-side prep: fold BN, transpose/pad/shard weights, pack smalls."""
    x = np.asarray(x, f32)
    h = np.asarray(h_rnn, f32).reshape(H)
    h_norm = np.asarray(h_norm, f32)

    # row5 (same on all cores)
    L4 = np.array([(1.0 - ALPHA) * ALPHA ** (4 - t) for t in range(5)], f32)
    row5 = np.zeros((5, R5_COLS), f32)
    row5[:, :IN] = x.reshape(5, IN)
    row5[0, IN:IN + F] = h_norm
    row5[:, IN + F] = L4
    row5[0, IN + F + 1] = ALPHA ** 5

    # fc1 + bn1 folded
    s1 = (np.asarray(g1, f32) / np.sqrt(np.asarray(rv1, f32) + BN_EPS))
    W1s = np.asarray(W1, f32) * s1[:, None]
    c1 = (np.asarray(b1, f32) - np.asarray(rm1, f32)) * s1 + np.asarray(beta1, f32)
    W1sT = np.zeros((K1, K1), f32)
    W1sT[:IN, :IN] = W1s.T
    w1t_img = _img(W1sT, 3, K1)
    c1_pf = _pf(np.pad(c1, (0, K1 - IN)), 3)

    x4d50 = np.pad(x[O - 1].reshape(IN) / 50.0, (0, K1 - IN)).astype(f32)
    x4_pf = _pf(x4d50, 3)

    # fc2 + bn2 folded
    s2 = (np.asarray(g2, f32) / np.sqrt(np.asarray(rv2, f32) + BN_EPS))
    W2s = np.asarray(W2, f32) * s2[:, None]
    c2 = (np.asarray(b2, f32) - np.asarray(rm2, f32)) * s2 + np.asarray(beta2, f32)
    c2_pf = _pf(c2 / NCORES, 16)
    hpf = _pf(h, 16)

    bsum = np.asarray(b_ih, f32) + np.asarray(b_hh, f32)
    W_ih = np.asarray(W_ih, f32)
    W_hh = np.asarray(W_hh, f32)
    Wout = np.asarray(Wout, f32)
    bout = np.asarray(bout, f32)
    wob = Wout.reshape(O, F, 2, H)
    bob = bout.reshape(O, F, 2)

    # +-1 reduction matrices for the final complex-mul (same on all cores)
    Mre = np.zeros((256, 2 * FB), f32)
    Mim = np.zeros((256, 2 * FB), f32)
    for t in range(WOC):
        o_, gci = divmod(t, 2 * FB)
        g, ci = divmod(gci, 2)
        Mre[t, 2 * g] = 1.0 if ci == 0 else -1.0
        Mim[t, 2 * g + 1] = 1.0
    mre_img = _img(Mre, 2, 2 * FB)
    mim_img = _img(Mim, 2, 2 * FB)

    in_maps = []
    for c in range(NCORES):
        gr = np.arange(PCH * c, PCH * (c + 1))
        idx = np.concatenate([gr, H + gr, 2 * H + gr])
        WihT = np.zeros((K1, GR), f32)
        WihT[:IN, :] = W_ih[idx, :].T
        wih_img = _img(WihT, 3, GR).astype(bf16)
        whh_img = _img(np.ascontiguousarray(W_hh[idx, :].T), 16, GR).astype(bf16)

        brz_pf = _pf(np.concatenate([bsum[gr], bsum[H + gr]]), 4)
        bin_pf = _pf(np.asarray(b_ih, f32)[2 * H + gr], 2)
        bhn_pf = _pf(np.asarray(b_hh, f32)[2 * H + gr], 2)
        hown_pf = _pf(h[gr], 2)

        w2_img = _img(np.ascontiguousarray(W2s[:, gr].T), 2, H).astype(bf16)

        # Wout rows for this core's bins, o-major
        WoT = np.zeros((H, WOC), f32)
        bo_c = np.zeros(WOC, f32)
        x2v = np.zeros(256, f32)
        xs2v = np.zeros(256, f32)
        f0 = FB * c
        for o_ in range(O):
            for g in range(FB):
                f = f0 + g
                if f >= F:
                    continue
                for ci in range(2):
                    t = o_ * 2 * FB + 2 * g + ci
                    WoT[:, t] = wob[o_, f, ci, :]
                    bo_c[t] = bob[o_, f, ci]
                    x2v[t] = 2.0 * x[o_, f, ci]
                    xs2v[t] = 2.0 * x[o_, f, 1 - ci]
        wot_img = _img(WoT, 16, WOC).astype(bf16)
        bout_pf = _pf(np.pad(bo_c, (0, 256 - WOC)), 2)

        smalls = np.zeros((128, SM_COLS), f32)

        def put(name, arr):
            o_, w = _SM_LAYOUT[name]
            smalls[:, o_:o_ + w] = arr

        put("x4", x4_pf)
        put("c1", c1_pf)
        put("brz", brz_pf)
        put("bin", bin_pf)
        put("bhn", bhn_pf)
        put("hpf", hpf)
        put("hown", hown_pf)
        put("c2", c2_pf)
        put("bout", bout_pf)
        put("x2", _pf(x2v, 2))
        put("xs2", _pf(xs2v, 2))
        put("mre", mre_img)
        put("mim", mim_img)

        in_maps.append({
            "row5": row5, "smalls": smalls, "w1t": w1t_img,
            "wih": wih_img, "whh": whh_img, "w2t": w2_img, "wot": wot_img,
            "h16": hpf.astype(bf16),
        })
    return in_maps


def assemble(results):
    out_full = np.concatenate(
        [results[c]["o_out"].ravel() for c in range(NCORES)])[:IN]
    out = out_full.reshape(F, 2).astype(f32)
    h_norm_new = results[0]["o_hn"].reshape(F).astype(f32)
    h_new = np.concatenate(
        [results[c]["o_hnew"].T.ravel() for c in range(NCORES)])
    return out, h_norm_new, h_new.reshape(1, 1, H).astype(f32)


_NC_CACHE = [None]


def run(inputs, trace=False, tmpdir=None):
    if _NC_CACHE[0] is None:
        _NC_CACHE[0] = build_nc()
    nc = _NC_CACHE[0]
    in_maps = prep_inputs(**inputs)
    res = run_bass_kernel_spmd(nc, in_maps, core_ids=list(range(NCORES)),
                               trace=trace, tmpdir=tmpdir)
    return assemble(res.results), res


def kernel(**inputs):
    (out, h_norm_new, h_new), _ = run(inputs)
    return out, h_norm_new, h_new


# revision 29
# speedup vs baseline: 1.1415x; 1.0275x over previous
"""CLCNet streaming step on 8 trn2 NeuronCores (Bass/Tile).

Strategy: tensor-parallel over 8 cores.
  - stage 0/1 (magnitude scan, 322x322 fc1+bn+relu) replicated on all cores
    (W1 is tiny; replication avoids a collective).
  - GRU: row-shard W_ih/W_hh (768 gate-rows per core = 256 rows of each of
    r/z/n).  Each core produces its 256-row shard of h_new.
  - fc2+bn: column-shard W2 by the core's h_new slice -> partial y2 [2048];
    one 8KB AllGather exchanges the partials (faster than ncfw AllReduce at
    this size) and each core tree-sums them in fp32 on DVE; relu replicated.
  - fc_out: row-shard Wout by frequency bins (21 bins/core, padded to 168);
    the final complex-multiply + sum over clc_order is done per-core with
    two constant +-1 matrices on the PE, so each core owns its bins' output.
All BN affines are folded into the weight matrices host-side; weights are
passed pre-transposed in SBUF-image layout ([128, k_chunks*M]) so every DMA
is a natural [128, N] transfer.
"""

import ml_dtypes
import numpy as np

bf16 = np.float16

import concourse.bacc as bacc
import concourse.tile as tile
import concourse.mybir as mybir
from concourse.bass_utils import run_bass_kernel_spmd

EPS = 1e-8
ALPHA = 0.99
OUT_ACT_F = 2.0
BN_EPS = 1e-5
O, F, H = 5, 161, 2048
IN = 2 * F          # 322
NCORES = 8
PCH = H // NCORES   # 256 = per-core h slice
GR = 3 * PCH        # 768 = per-core gate rows
K1 = 384            # 322 padded to 3*128
FB = 21             # freq bins per core (8*21 = 168 >= 161)
WOC = O * 2 * FB    # 210 = per-core Wout rows
DT = mybir.dt.float32
BT = mybir.dt.float16
f32 = np.float32

# column layout of the packed "smalls" [128, 222] input
_SM_LAYOUT = {}
_off = 0
for _name, _w in [("x4", 3), ("c1", 3), ("brz", 4), ("bin", 2), ("bhn", 2),
                  ("hpf", 16), ("hown", 2), ("c2", 16), ("bout", 2),
                  ("x2", 2), ("xs2", 2), ("mre", 84), ("mim", 84)]:
    _SM_LAYOUT[_name] = (_off, _w)
    _off += _w
SM_COLS = _off  # 222

# row5 packed [5, 485]: x5 [5,322] | hnr [1,161] | L4 [5,1] | a5 [1,1]
R5_COLS = 322 + 161 + 1 + 1


def _img(mat_t, kchunks, mcols):
    """[kchunks*128, mcols] -> SBUF image [128, kchunks*mcols]."""
    return np.ascontiguousarray(
        mat_t.reshape(kchunks, 128, mcols).transpose(1, 0, 2)
        .reshape(128, kchunks * mcols))


def _pf(vec, cols):
    """flat [cols*128] -> [128, cols] partition-first image."""
    return np.ascontiguousarray(vec.reshape(cols, 128).T)


def build_nc():
    nc = bacc.Bacc("TRN2", target_bir_lowering=False, debug=False,
                   num_devices=NCORES)

    i_row5 = nc.dram_tensor("row5", [5, R5_COLS], DT, kind="ExternalInput")
    i_sm = nc.dram_tensor("smalls", [128, SM_COLS], DT, kind="ExternalInput")
    i_w1 = nc.dram_tensor("w1t", [128, 3 * K1], DT, kind="ExternalInput")
    i_wih = nc.dram_tensor("wih", [128, 3 * GR], BT, kind="ExternalInput")
    i_whh = nc.dram_tensor("whh", [128, 16 * GR], BT, kind="ExternalInput")
    i_w2 = nc.dram_tensor("w2t", [128, 2 * H], BT, kind="ExternalInput")
    i_wot = nc.dram_tensor("wot", [128, 16 * WOC], BT, kind="ExternalInput")
    i_h16 = nc.dram_tensor("h16", [128, 16], BT, kind="ExternalInput")

    o_hn = nc.dram_tensor("o_hn", [1, F], DT, kind="ExternalOutput")
    o_hnew = nc.dram_tensor("o_hnew", [128, 2], DT, kind="ExternalOutput")
    o_out = nc.dram_tensor("o_out", [2 * FB, 1], DT, kind="ExternalOutput")

    AF = mybir.ActivationFunctionType

    with tile.TileContext(nc) as tc:
        with (
            tc.tile_pool(name="sb", bufs=1) as sb,
            tc.tile_pool(name="ps", bufs=1, space="PSUM") as ps,
            tc.tile_pool(name="dram", bufs=1, space="DRAM") as dram,
        ):
            # ---- input DMAs (emission order ~ priority) ----
            row5 = sb.tile([5, R5_COLS], DT)
            nc.sync.dma_start(row5[:], i_row5[:])
            sm = sb.tile([128, SM_COLS], DT)
            nc.sync.dma_start(sm[:], i_sm[:])
            w1t = sb.tile([128, 3 * K1], DT)
            nc.sync.dma_start(w1t[:], i_w1[:])
            wih = sb.tile([128, 3 * GR], BT)
            nc.sync.dma_start(wih[:], i_wih[:])
            whh = []
            for s in range(4):  # 4 slabs x 4 k-chunks
                t = sb.tile([128, 4 * GR], BT, tag=f"whh{s}")
                nc.sync.dma_start(t[:], i_whh[:, s * 4 * GR:(s + 1) * 4 * GR])
                whh.append(t)
            w2 = []
            for s in range(2):
                t = sb.tile([128, H], BT, tag=f"w2{s}")
                nc.sync.dma_start(t[:], i_w2[:, s * H:(s + 1) * H])
                w2.append(t)
            wot = []
            for s in range(2):
                t = sb.tile([128, 8 * WOC], BT, tag=f"wot{s}")
                nc.sync.dma_start(t[:], i_wot[:, s * 8 * WOC:(s + 1) * 8 * WOC])
                wot.append(t)

            hpf16 = sb.tile([128, 16], BT)
            nc.sync.dma_start(hpf16[:], i_h16[:])

            def smv(name):
                o, w = _SM_LAYOUT[name]
                return sm[:, o:o + w]

            x5 = row5[:, 0:IN]                 # [5,322]
            hnr = row5[0:1, IN:IN + F]         # [1,161]
            L4v = row5[:, IN + F:IN + F + 1]   # [5,1]
            a5v = row5[0:1, IN + F + 1:IN + F + 2]  # [1,1]

            zb = sb.tile([128, 1], DT)         # zero bias for activations
            nc.vector.memset(zb[:], 0.0)
            eps5 = sb.tile([5, 1], DT)
            nc.vector.memset(eps5[:], EPS)

            # ---- stage 0: magnitude + exp-decay scan (as 5x5 matmul) ----
            sq = sb.tile([5, IN], DT)
            nc.vector.tensor_mul(sq[:], x5, x5)
            sqv = sq[:].rearrange("p (f two) -> p f two", two=2)
            m2 = sb.tile([5, F], DT)
            nc.vector.tensor_add(m2[:], sqv[:, :, 0], sqv[:, :, 1])
            xm = sb.tile([5, F], DT)
            nc.scalar.activation(xm[:], m2[:], AF.Sqrt, bias=eps5[:])

            S4 = ps.tile([1, F], DT, tag="b0")
            nc.tensor.matmul(S4[:], L4v, xm[:], start=True, stop=False)
            nc.tensor.matmul(S4[:], a5v, hnr, start=False, stop=True)

            s4_sb = sb.tile([1, F], DT)
            nc.vector.tensor_copy(s4_sb[:], S4[:])
            nc.sync.dma_start(o_hn[:], s4_sb[:])

            den = sb.tile([1, F], DT)
            nc.vector.tensor_scalar_add(den[:], s4_sb[:], EPS)
            rec = sb.tile([1, F], DT)
            nc.vector.reciprocal(rec[:], den[:])
            rexp = sb.tile([1, IN], DT)
            rexpv = rexp[:].rearrange("p (f two) -> p f two", two=2)
            nc.vector.tensor_copy(rexpv[:, :, 0], rec[:])
            nc.vector.tensor_copy(rexpv[:, :, 1], rec[:])

            ones11 = sb.tile([1, 1], DT)
            nc.vector.memset(ones11[:], 1.0)

            # transpose rexp (free layout) into partition layout via PE
            vt = ps.tile([128, 3], DT, tag="b1")
            nc.tensor.matmul(vt[:, 0:1], rexp[0:1, 0:128], ones11[:],
                             start=True, stop=True)
            nc.tensor.matmul(vt[:, 1:2], rexp[0:1, 128:256], ones11[:],
                             start=True, stop=True)
            nc.tensor.matmul(vt[0:66, 2:3], rexp[0:1, 256:322], ones11[:],
                             start=True, stop=True)

            v_pf = sb.tile([128, 3], DT)
            nc.vector.memset(v_pf[:], 0.0)
            nc.vector.tensor_mul(v_pf[:, 0:2], vt[:, 0:2], smv("x4")[:, 0:2])
            nc.vector.tensor_mul(v_pf[0:66, 2:3], vt[0:66, 2:3],
                                 smv("x4")[0:66, 2:3])

            # ---- stage 1: fc1+bn+relu (replicated) ----
            # m-chunk outer: sequential accumulation groups share one bank
            y1p = ps.tile([128, 3], DT, tag="b2")
            for i in range(3):
                for j in range(3):
                    nc.tensor.matmul(
                        y1p[:, i:i + 1],
                        w1t[:, K1 * j + 128 * i: K1 * j + 128 * (i + 1)],
                        v_pf[:, j:j + 1], start=(j == 0), stop=(j == 2))
            y1 = sb.tile([128, 3], BT)
            for i in range(3):
                nc.scalar.activation(y1[:, i:i + 1], y1p[:, i:i + 1], AF.Relu,
                                     bias=smv("c1")[:, i:i + 1])

            # ---- stage 2: GRU (row-sharded) ----
            # one PSUM bank per concurrent accumulation group; groups start
            # with the streaming W_hh part so PE can run before y1 is ready
            grz = [ps.tile([128, 1], DT, tag=f"b{3 + i}", name=f"grz{i}")
                   for i in range(4)]
            gin = ps.tile([128, 2], DT, tag="b7")    # n gate, ih part
            ghn = [ps.tile([128, 1], DT, tag=t, name=f"ghn{t}")
                   for t in ("b0", "b1")]
            for j2 in range(16):    # k-chunks of h
                s, jj = divmod(j2, 4)
                for mi in range(4):
                    nc.tensor.matmul(
                        grz[mi][:],
                        whh[s][:, GR * jj + 128 * mi: GR * jj + 128 * (mi + 1)],
                        hpf16[:, j2:j2 + 1], start=(j2 == 0), stop=False)
                for m2_ in range(2):
                    mi = 4 + m2_
                    nc.tensor.matmul(
                        ghn[m2_][:],
                        whh[s][:, GR * jj + 128 * mi: GR * jj + 128 * (mi + 1)],
                        hpf16[:, j2:j2 + 1], start=(j2 == 0), stop=(j2 == 15))
            for j in range(3):      # k-chunks of y1 (finish the r/z groups)
                for mi in range(4):
                    nc.tensor.matmul(
                        grz[mi][:],
                        wih[:, GR * j + 128 * mi: GR * j + 128 * (mi + 1)],
                        y1[:, j:j + 1], start=False, stop=(j == 2))
            for m2_ in range(2):    # n gate ih part: sequential groups
                mi = 4 + m2_
                for j in range(3):
                    nc.tensor.matmul(
                        gin[:, m2_:m2_ + 1],
                        wih[:, GR * j + 128 * mi: GR * j + 128 * (mi + 1)],
                        y1[:, j:j + 1], start=(j == 0), stop=(j == 2))

            r_sb = sb.tile([128, 2], DT)
            z_sb = sb.tile([128, 2], DT)
            for i in range(2):
                nc.scalar.activation(r_sb[:, i:i + 1], grz[i][:],
                                     AF.Sigmoid, bias=smv("brz")[:, i:i + 1])
                nc.scalar.activation(z_sb[:, i:i + 1], grz[2 + i][:],
                                     AF.Sigmoid, bias=smv("brz")[:, 2 + i:3 + i])
            t1 = sb.tile([128, 2], DT)
            t2 = sb.tile([128, 2], DT)
            for i in range(2):
                nc.vector.tensor_scalar_add(t1[:, i:i + 1], gin[:, i:i + 1],
                                            smv("bin")[:, i:i + 1])
                nc.vector.tensor_scalar_add(t2[:, i:i + 1], ghn[i][:],
                                            smv("bhn")[:, i:i + 1])
            t3 = sb.tile([128, 2], DT)
            nc.vector.tensor_mul(t3[:], r_sb[:], t2[:])
            t4 = sb.tile([128, 2], DT)
            nc.vector.tensor_add(t4[:], t1[:], t3[:])
            n_sb = sb.tile([128, 2], DT)
            nc.scalar.activation(n_sb[:], t4[:], AF.Tanh, bias=zb[:])
            t5 = sb.tile([128, 2], DT)
            nc.vector.tensor_sub(t5[:], smv("hown"), n_sb[:])
            t6 = sb.tile([128, 2], DT)
            nc.vector.tensor_mul(t6[:], z_sb[:], t5[:])
            hn_sb = sb.tile([128, 2], DT)
            nc.vector.tensor_add(hn_sb[:], n_sb[:], t6[:])
            nc.sync.dma_start(o_hnew[:], hn_sb[:])
            hn16 = sb.tile([128, 2], BT)
            nc.vector.tensor_copy(hn16[:], hn_sb[:])

            # ---- stage 3: fc2 partial (column-sharded) + AllReduce ----
            # m-chunk outer: 16 sequential groups in one bank
            y2p = ps.tile([128, 16], DT, tag="b2")
            for mi in range(16):
                for j in range(2):
                    nc.tensor.matmul(
                        y2p[:, mi:mi + 1],
                        w2[j][:, 128 * mi:128 * (mi + 1)],
                        hn16[:, j:j + 1], start=(j == 0), stop=(j == 1))
            y2sb = sb.tile([128, 16], DT)
            nc.vector.tensor_add(y2sb[:], y2p[:], smv("c2"))

            # AllGather the 8 partials (copy-only, ~2x faster than ncfw
            # AllReduce at this size), then tree-sum locally in fp32 on DVE
            cc_in = dram.tile([128, 16], DT)
            cc_out = dram.tile([NCORES * 128, 16], DT, addr_space="Shared")
            nc.sync.dma_start(cc_in[:], y2sb[:])
            nc.gpsimd.collective_compute(
                "AllGather", mybir.AluOpType.bypass,
                replica_groups=[list(range(NCORES))],
                ins=[cc_in[:].opt()], outs=[cc_out[:].opt()])
            y2all = sb.tile([128, NCORES * 16], DT)
            nc.sync.dma_start(
                y2all[:].rearrange("p (r f) -> p r f", r=NCORES),
                cc_out[:].rearrange("(r p) f -> p r f", p=128))
            ts1 = sb.tile([128, 64], DT)
            nc.vector.tensor_add(ts1[:], y2all[:, 0:64], y2all[:, 64:128])
            ts2 = sb.tile([128, 32], DT)
            nc.vector.tensor_add(ts2[:], ts1[:, 0:32], ts1[:, 32:64])
            y2f = sb.tile([128, 16], DT)
            nc.vector.tensor_add(y2f[:], ts2[:, 0:16], ts2[:, 16:32])

            u_sb = sb.tile([128, 16], BT)
            nc.scalar.activation(u_sb[:], y2f[:], AF.Relu, bias=zb[:])

            # ---- stage 4: fc_out (bin-sharded) + tanh ----
            # two PSUM banks so col0's tanh/products/osum overlap col1's
            # matmul stretch (single-bank serializes the read behind both)
            cpa = ps.tile([128, 1], DT, tag="b3")
            cpb = ps.tile([128, 1], DT, tag="b5")
            for j in range(16):
                s, jj = divmod(j, 8)
                base = WOC * jj
                nc.tensor.matmul(cpa[:],
                                 wot[s][:, base:base + 128],
                                 u_sb[:, j:j + 1], start=(j == 0), stop=(j == 15))
            for j in range(16):
                s, jj = divmod(j, 8)
                base = WOC * jj
                nc.tensor.matmul(cpb[0:82, :],
                                 wot[s][:, base + 128:base + WOC],
                                 u_sb[:, j:j + 1], start=(j == 0), stop=(j == 15))
            coef = sb.tile([128, 2], DT)
            nc.vector.memset(coef[:], 0.0)
            nc.scalar.activation(coef[:, 0:1], cpa[:], AF.Tanh,
                                 bias=smv("bout")[:, 0:1])
            p1 = sb.tile([128, 2], DT)
            p2 = sb.tile([128, 2], DT)
            nc.vector.tensor_mul(p1[:, 0:1], smv("x2")[:, 0:1], coef[:, 0:1])
            nc.vector.tensor_mul(p2[:, 0:1], smv("xs2")[:, 0:1], coef[:, 0:1])
            nc.scalar.activation(coef[0:82, 1:2], cpb[0:82, :], AF.Tanh,
                                 bias=smv("bout")[0:82, 1:2])
            nc.vector.tensor_mul(p1[:, 1:2], smv("x2")[:, 1:2], coef[:, 1:2])
            nc.vector.tensor_mul(p2[:, 1:2], smv("xs2")[:, 1:2], coef[:, 1:2])

            # ---- final: complex multiply + sum over clc_order ----
            osum = ps.tile([2 * FB, 1], DT, tag="b4")
            mre = smv("mre")
            mim = smv("mim")
            nc.tensor.matmul(osum[:], mre[:, 0:42], p1[:, 0:1],
                             start=True, stop=False)
            nc.tensor.matmul(osum[:], mim[:, 0:42], p2[:, 0:1],
                             start=False, stop=False)
            nc.tensor.matmul(osum[:], mre[:, 42:84], p1[:, 1:2],
                             start=False, stop=False)
            nc.tensor.matmul(osum[:], mim[:, 42:84], p2[:, 1:2],
                             start=False, stop=True)
            ot = sb.tile([2 * FB, 1], DT)
            nc.scalar.activation(ot[:], osum[:], AF.Copy)
            nc.sync.dma_start(o_out[:], ot[:])

    nc.compile()
    return nc


def prep_inputs(x, h_norm, h_rnn, W1, b1, g1, beta1, rm1, rv1,
                W_ih, W_hh, b_ih, b_hh, W2, b2, g2, beta2, rm2, rv2,
                Wout, bout):
    """Host-side prep: fold BN, transpose/pad/shard weights, pack smalls."""
    x = np.asarray(x, f32)
    h = np.asarray(h_rnn, f32).reshape(H)
    h_norm = np.asarray(h_norm, f32)

    # row5 (same on all cores)
    L4 = np.array([(1.0 - ALPHA) * ALPHA ** (4 - t) for t in range(5)], f32)
    row5 = np.zeros((5, R5_COLS), f32)
    row5[:, :IN] = x.reshape(5, IN)
    row5[0, IN:IN + F] = h_norm
    row5[:, IN + F] = L4
    row5[0, IN + F + 1] = ALPHA ** 5

    # fc1 + bn1 folded
    s1 = (np.asarray(g1, f32) / np.sqrt(np.asarray(rv1, f32) + BN_EPS))
    W1s = np.asarray(W1, f32) * s1[:, None]
    c1 = (np.asarray(b1, f32) - np.asarray(rm1, f32)) * s1 + np.asarray(beta1, f32)
    W1sT = np.zeros((K1, K1), f32)
    W1sT[:IN, :IN] = W1s.T
    w1t_img = _img(W1sT, 3, K1)
    c1_pf = _pf(np.pad(c1, (0, K1 - IN)), 3)

    x4d50 = np.pad(x[O - 1].reshape(IN) / 50.0, (0, K1 - IN)).astype(f32)
    x4_pf = _pf(x4d50, 3)

    # fc2 + bn2 folded
    s2 = (np.asarray(g2, f32) / np.sqrt(np.asarray(rv2, f32) + BN_EPS))
    W2s = np.asarray(W2, f32) * s2[:, None]
    c2 = (np.asarray(b2, f32) - np.asarray(rm2, f32)) * s2 + np.asarray(beta2, f32)
    c2_pf = _pf(c2 / NCORES, 16)
    hpf = _pf(h, 16)

    bsum = np.asarray(b_ih, f32) + np.asarray(b_hh, f32)
    W_ih = np.asarray(W_ih, f32)
    W_hh = np.asarray(W_hh, f32)
    Wout = np.asarray(Wout, f32)
    bout = np.asarray(bout, f32)
    wob = Wout.reshape(O, F, 2, H)
    bob = bout.reshape(O, F, 2)

    # +-1 reduction matrices for the final complex-mul (same on all cores)
    Mre = np.zeros((256, 2 * FB), f32)
    Mim = np.zeros((256, 2 * FB), f32)
    for t in range(WOC):
        o_, gci = divmod(t, 2 * FB)
        g, ci = divmod(gci, 2)
        Mre[t, 2 * g] = 1.0 if ci == 0 else -1.0
        Mim[t, 2 * g + 1] = 1.0
    mre_img = _img(Mre, 2, 2 * FB)
    mim_img = _img(Mim, 2, 2 * FB)

    in_maps = []
    for c in range(NCORES):
        gr = np.arange(PCH * c, PCH * (c + 1))
        idx = np.concatenate([gr, H + gr, 2 * H + gr])
        WihT = np.zeros((K1, GR), f32)
        WihT[:IN, :] = W_ih[idx, :].T
        wih_img = _img(WihT, 3, GR).astype(bf16)
        whh_img = _img(np.ascontiguousarray(W_hh[idx, :].T), 16, GR).astype(bf16)

        brz_pf = _pf(np.concatenate([bsum[gr], bsum[H + gr]]), 4)
        bin_pf = _pf(np.asarray(b_ih, f32)[2 * H + gr], 2)
        bhn_pf = _pf(np.asarray(b_hh, f32)[2 * H + gr], 2)
        hown_pf = _pf(h[gr], 2)

        w2_img = _img(np.ascontiguousarray(W2s[:, gr].T), 2, H).astype(bf16)

        # Wout rows for this core's bins, o-major
        WoT = np.zeros((H, WOC), f32)
        bo_c = np.zeros(WOC, f32)
        x2v = np.zeros(256, f32)
        xs2v = np.zeros(256, f32)
        f0 = FB * c
        for o_ in range(O):
            for g in range(FB):
                f = f0 + g
                if f >= F:
                    continue
                for ci in range(2):
                    t = o_ * 2 * FB + 2 * g + ci
                    WoT[:, t] = wob[o_, f, ci, :]
                    bo_c[t] = bob[o_, f, ci]
                    x2v[t] = 2.0 * x[o_, f, ci]
                    xs2v[t] = 2.0 * x[o_, f, 1 - ci]
        wot_img = _img(WoT, 16, WOC).astype(bf16)
        bout_pf = _pf(np.pad(bo_c, (0, 256 - WOC)), 2)

        smalls = np.zeros((128, SM_COLS), f32)

        def put(name, arr):
            o_, w = _SM_LAYOUT[name]
            smalls[:, o_:o_ + w] = arr

        put("x4", x4_pf)
        put("c1", c1_pf)
        put("brz", brz_pf)
        put("bin", bin_pf)
        put("bhn", bhn_pf)
        put("hpf", hpf)
        put("hown", hown_pf)
        put("c2", c2_pf)
        put("bout", bout_pf)
        put("x2", _pf(x2v, 2))
        put("xs2", _pf(xs2v, 2))
        put("mre", mre_img)
        put("mim", mim_img)

        in_maps.append({
            "row5": row5, "smalls": smalls, "w1t": w1t_img,
            "wih": wih_img, "whh": whh_img, "w2t": w2_img, "wot": wot_img,
            "h16": hpf.astype(bf16),
        })
    return in_maps


def assemble(results):
    out_full = np.concatenate(
        [results[c]["o_out"].ravel() for c in range(NCORES)])[:IN]
    out = out_full.reshape(F, 2).astype(f32)
    h_norm_new = results[0]["o_hn"].reshape(F).astype(f32)
    h_new = np.concatenate(
        [results[c]["o_hnew"].T.ravel() for c in range(NCORES)])
    return out, h_norm_new, h_new.reshape(1, 1, H).astype(f32)


_NC_CACHE = [None]


def run(inputs, trace=False, tmpdir=None):
    if _NC_CACHE[0] is None:
        _NC_CACHE[0] = build_nc()
    nc = _NC_CACHE[0]
    in_maps = prep_inputs(**inputs)
    res = run_bass_kernel_spmd(nc, in_maps, core_ids=list(range(NCORES)),
                               trace=trace, tmpdir=tmpdir)
    return assemble(res.results), res


def kernel(**inputs):
    (out, h_norm_new, h_new), _ = run(inputs)
    return out, h_norm_new, h_new


# revision 30
# speedup vs baseline: 1.2799x; 1.1212x over previous
"""CLCNet streaming step on 8 trn2 NeuronCores (Bass/Tile).

Strategy: tensor-parallel over 8 cores.
  - stage 0/1 (magnitude scan, 322x322 fc1+bn+relu) replicated on all cores
    (W1 is tiny; replication avoids a collective).
  - GRU: row-shard W_ih/W_hh (768 gate-rows per core = 256 rows of each of
    r/z/n).  Each core produces its 256-row shard of h_new.
  - fc2+bn: column-shard W2 by the core's h_new slice -> partial y2 [2048];
    one 8KB AllGather exchanges the partials (faster than ncfw AllReduce at
    this size) and each core tree-sums them in fp32 on DVE; relu replicated.
  - fc_out: row-shard Wout by frequency bins (21 bins/core, padded to 168);
    the final complex-multiply + sum over clc_order is done per-core with
    two constant +-1 matrices on the PE, so each core owns its bins' output.
All BN affines are folded into the weight matrices host-side; weights are
passed pre-transposed in SBUF-image layout ([128, k_chunks*M]) so every DMA
is a natural [128, N] transfer.
"""

import ml_dtypes
import numpy as np

bf16 = np.float16

import concourse.bacc as bacc
import concourse.tile as tile
import concourse.mybir as mybir
from concourse.bass_utils import run_bass_kernel_spmd

EPS = 1e-8
ALPHA = 0.99
OUT_ACT_F = 2.0
BN_EPS = 1e-5
O, F, H = 5, 161, 2048
IN = 2 * F          # 322
NCORES = 8
PCH = H // NCORES   # 256 = per-core h slice
GR = 3 * PCH        # 768 = per-core gate rows
K1 = 384            # 322 padded to 3*128
FB = 21             # freq bins per core (8*21 = 168 >= 161)
WOC = O * 2 * FB    # 210 = per-core Wout rows
DT = mybir.dt.float32
BT = mybir.dt.float16
f32 = np.float32

# column layout of the packed "smalls" [128, 222] input
_SM_LAYOUT = {}
_off = 0
for _name, _w in [("x4", 3), ("c1", 3), ("brz", 4), ("bin", 2), ("bhn", 2),
                  ("hpf", 16), ("hown", 2), ("c2", 16), ("bout", 2),
                  ("x2", 2), ("xs2", 2), ("mre", 84), ("mim", 84)]:
    _SM_LAYOUT[_name] = (_off, _w)
    _off += _w
SM_COLS = _off  # 222

# row5 packed [5, 485]: x5 [5,322] | hnr [1,161] | L4 [5,1] | a5 [1,1]
R5_COLS = 322 + 161 + 1 + 1


def _img(mat_t, kchunks, mcols):
    """[kchunks*128, mcols] -> SBUF image [128, kchunks*mcols]."""
    return np.ascontiguousarray(
        mat_t.reshape(kchunks, 128, mcols).transpose(1, 0, 2)
        .reshape(128, kchunks * mcols))


def _pf(vec, cols):
    """flat [cols*128] -> [128, cols] partition-first image."""
    return np.ascontiguousarray(vec.reshape(cols, 128).T)


def build_nc():
    nc = bacc.Bacc("TRN2", target_bir_lowering=False, debug=False,
                   num_devices=NCORES)

    i_row5 = nc.dram_tensor("row5", [5, R5_COLS], DT, kind="ExternalInput")
    i_sm = nc.dram_tensor("smalls", [128, SM_COLS], DT, kind="ExternalInput")
    i_w1 = nc.dram_tensor("w1t", [128, 3 * K1], DT, kind="ExternalInput")
    i_wih = nc.dram_tensor("wih", [128, 3 * GR], BT, kind="ExternalInput")
    i_whh = nc.dram_tensor("whh", [128, 16 * GR], BT, kind="ExternalInput")
    i_w2 = nc.dram_tensor("w2t", [128, 2 * H], BT, kind="ExternalInput")
    i_wot = nc.dram_tensor("wot", [128, 16 * WOC], BT, kind="ExternalInput")
    i_h16 = nc.dram_tensor("h16", [128, 16], BT, kind="ExternalInput")

    o_hn = nc.dram_tensor("o_hn", [1, F], DT, kind="ExternalOutput")
    o_hnew = nc.dram_tensor("o_hnew", [128, 2], DT, kind="ExternalOutput")
    o_out = nc.dram_tensor("o_out", [2 * FB, 1], DT, kind="ExternalOutput")

    AF = mybir.ActivationFunctionType

    with tile.TileContext(nc) as tc:
        with (
            tc.tile_pool(name="sb", bufs=1) as sb,
            tc.tile_pool(name="ps", bufs=1, space="PSUM") as ps,
            tc.tile_pool(name="dram", bufs=1, space="DRAM") as dram,
        ):
            # ---- input DMAs, split across both HWDGE rings ----
            # sync ring: only the GRU-critical W_hh stream (first in, alone)
            whh = []
            for s in range(4):  # 4 slabs x 4 k-chunks
                t = sb.tile([128, 4 * GR], BT, tag=f"whh{s}")
                nc.sync.dma_start(t[:], i_whh[:, s * 4 * GR:(s + 1) * 4 * GR])
                whh.append(t)
            # scalar ring: everything else, in deadline order (wot last --
            # it is only needed after the collective)
            row5 = sb.tile([5, R5_COLS], DT)
            nc.scalar.dma_start(row5[:], i_row5[:])
            sm = sb.tile([128, SM_COLS], DT)
            nc.scalar.dma_start(sm[:], i_sm[:])
            hpf16 = sb.tile([128, 16], BT)
            nc.scalar.dma_start(hpf16[:], i_h16[:])
            wih = sb.tile([128, 3 * GR], BT)
            nc.scalar.dma_start(wih[:], i_wih[:])
            w1t = sb.tile([128, 3 * K1], DT)
            nc.scalar.dma_start(w1t[:], i_w1[:])
            w2 = []
            for s in range(2):
                t = sb.tile([128, H], BT, tag=f"w2{s}")
                nc.scalar.dma_start(t[:], i_w2[:, s * H:(s + 1) * H])
                w2.append(t)
            wot = []
            for s in range(2):
                t = sb.tile([128, 8 * WOC], BT, tag=f"wot{s}")
                nc.scalar.dma_start(t[:], i_wot[:, s * 8 * WOC:(s + 1) * 8 * WOC])
                wot.append(t)

            def smv(name):
                o, w = _SM_LAYOUT[name]
                return sm[:, o:o + w]

            x5 = row5[:, 0:IN]                 # [5,322]
            hnr = row5[0:1, IN:IN + F]         # [1,161]
            L4v = row5[:, IN + F:IN + F + 1]   # [5,1]
            a5v = row5[0:1, IN + F + 1:IN + F + 2]  # [1,1]

            zb = sb.tile([128, 1], DT)         # zero bias for activations
            nc.vector.memset(zb[:], 0.0)
            eps5 = sb.tile([5, 1], DT)
            nc.vector.memset(eps5[:], EPS)

            # ---- stage 0: magnitude + exp-decay scan (as 5x5 matmul) ----
            sq = sb.tile([5, IN], DT)
            nc.vector.tensor_mul(sq[:], x5, x5)
            sqv = sq[:].rearrange("p (f two) -> p f two", two=2)
            m2 = sb.tile([5, F], DT)
            nc.vector.tensor_add(m2[:], sqv[:, :, 0], sqv[:, :, 1])
            xm = sb.tile([5, F], DT)
            nc.scalar.activation(xm[:], m2[:], AF.Sqrt, bias=eps5[:])

            S4 = ps.tile([1, F], DT, tag="b0")
            nc.tensor.matmul(S4[:], L4v, xm[:], start=True, stop=False)
            nc.tensor.matmul(S4[:], a5v, hnr, start=False, stop=True)

            s4_sb = sb.tile([1, F], DT)
            nc.vector.tensor_copy(s4_sb[:], S4[:])
            nc.sync.dma_start(o_hn[:], s4_sb[:])

            den = sb.tile([1, F], DT)
            nc.vector.tensor_scalar_add(den[:], s4_sb[:], EPS)
            rec = sb.tile([1, F], DT)
            nc.vector.reciprocal(rec[:], den[:])
            rexp = sb.tile([1, IN], DT)
            rexpv = rexp[:].rearrange("p (f two) -> p f two", two=2)
            nc.vector.tensor_copy(rexpv[:, :, 0], rec[:])
            nc.vector.tensor_copy(rexpv[:, :, 1], rec[:])

            ones11 = sb.tile([1, 1], DT)
            nc.vector.memset(ones11[:], 1.0)

            # transpose rexp (free layout) into partition layout via PE
            vt = ps.tile([128, 3], DT, tag="b1")
            nc.tensor.matmul(vt[:, 0:1], rexp[0:1, 0:128], ones11[:],
                             start=True, stop=True)
            nc.tensor.matmul(vt[:, 1:2], rexp[0:1, 128:256], ones11[:],
                             start=True, stop=True)
            nc.tensor.matmul(vt[0:66, 2:3], rexp[0:1, 256:322], ones11[:],
                             start=True, stop=True)

            v_pf = sb.tile([128, 3], DT)
            nc.vector.memset(v_pf[:], 0.0)
            nc.vector.tensor_mul(v_pf[:, 0:2], vt[:, 0:2], smv("x4")[:, 0:2])
            nc.vector.tensor_mul(v_pf[0:66, 2:3], vt[0:66, 2:3],
                                 smv("x4")[0:66, 2:3])

            # ---- stage 1: fc1+bn+relu (replicated) ----
            # m-chunk outer: sequential accumulation groups share one bank
            y1p = ps.tile([128, 3], DT, tag="b2")
            for i in range(3):
                for j in range(3):
                    nc.tensor.matmul(
                        y1p[:, i:i + 1],
                        w1t[:, K1 * j + 128 * i: K1 * j + 128 * (i + 1)],
                        v_pf[:, j:j + 1], start=(j == 0), stop=(j == 2))
            y1 = sb.tile([128, 3], BT)
            for i in range(3):
                nc.scalar.activation(y1[:, i:i + 1], y1p[:, i:i + 1], AF.Relu,
                                     bias=smv("c1")[:, i:i + 1])

            # ---- stage 2: GRU (row-sharded) ----
            # one PSUM bank per concurrent accumulation group; groups start
            # with the streaming W_hh part so PE can run before y1 is ready
            grz = [ps.tile([128, 1], DT, tag=f"b{3 + i}", name=f"grz{i}")
                   for i in range(4)]
            gin = ps.tile([128, 2], DT, tag="b7")    # n gate, ih part
            ghn = [ps.tile([128, 1], DT, tag=t, name=f"ghn{t}")
                   for t in ("b0", "b1")]
            for j2 in range(16):    # k-chunks of h
                s, jj = divmod(j2, 4)
                for mi in range(4):
                    nc.tensor.matmul(
                        grz[mi][:],
                        whh[s][:, GR * jj + 128 * mi: GR * jj + 128 * (mi + 1)],
                        hpf16[:, j2:j2 + 1], start=(j2 == 0), stop=False)
                for m2_ in range(2):
                    mi = 4 + m2_
                    nc.tensor.matmul(
                        ghn[m2_][:],
                        whh[s][:, GR * jj + 128 * mi: GR * jj + 128 * (mi + 1)],
                        hpf16[:, j2:j2 + 1], start=(j2 == 0), stop=(j2 == 15))
            for j in range(3):      # k-chunks of y1 (finish the r/z groups)
                for mi in range(4):
                    nc.tensor.matmul(
                        grz[mi][:],
                        wih[:, GR * j + 128 * mi: GR * j + 128 * (mi + 1)],
                        y1[:, j:j + 1], start=False, stop=(j == 2))
            for m2_ in range(2):    # n gate ih part: sequential groups
                mi = 4 + m2_
                for j in range(3):
                    nc.tensor.matmul(
                        gin[:, m2_:m2_ + 1],
                        wih[:, GR * j + 128 * mi: GR * j + 128 * (mi + 1)],
                        y1[:, j:j + 1], start=(j == 0), stop=(j == 2))

            r_sb = sb.tile([128, 2], DT)
            z_sb = sb.tile([128, 2], DT)
            for i in range(2):
                nc.scalar.activation(r_sb[:, i:i + 1], grz[i][:],
                                     AF.Sigmoid, bias=smv("brz")[:, i:i + 1])
                nc.scalar.activation(z_sb[:, i:i + 1], grz[2 + i][:],
                                     AF.Sigmoid, bias=smv("brz")[:, 2 + i:3 + i])
            t1 = sb.tile([128, 2], DT)
            t2 = sb.tile([128, 2], DT)
            for i in range(2):
                nc.vector.tensor_scalar_add(t1[:, i:i + 1], gin[:, i:i + 1],
                                            smv("bin")[:, i:i + 1])
                nc.vector.tensor_scalar_add(t2[:, i:i + 1], ghn[i][:],
                                            smv("bhn")[:, i:i + 1])
            t3 = sb.tile([128, 2], DT)
            nc.vector.tensor_mul(t3[:], r_sb[:], t2[:])
            t4 = sb.tile([128, 2], DT)
            nc.vector.tensor_add(t4[:], t1[:], t3[:])
            n_sb = sb.tile([128, 2], DT)
            nc.scalar.activation(n_sb[:], t4[:], AF.Tanh, bias=zb[:])
            t5 = sb.tile([128, 2], DT)
            nc.vector.tensor_sub(t5[:], smv("hown"), n_sb[:])
            t6 = sb.tile([128, 2], DT)
            nc.vector.tensor_mul(t6[:], z_sb[:], t5[:])
            hn_sb = sb.tile([128, 2], DT)
            nc.vector.tensor_add(hn_sb[:], n_sb[:], t6[:])
            nc.sync.dma_start(o_hnew[:], hn_sb[:])
            hn16 = sb.tile([128, 2], BT)
            nc.vector.tensor_copy(hn16[:], hn_sb[:])

            # ---- stage 3: fc2 partial (column-sharded) + AllReduce ----
            # m-chunk outer: 16 sequential groups in one bank
            y2p = ps.tile([128, 16], DT, tag="b2")
            for mi in range(16):
                for j in range(2):
                    nc.tensor.matmul(
                        y2p[:, mi:mi + 1],
                        w2[j][:, 128 * mi:128 * (mi + 1)],
                        hn16[:, j:j + 1], start=(j == 0), stop=(j == 1))
            y2sb = sb.tile([128, 16], DT)
            nc.vector.tensor_add(y2sb[:], y2p[:], smv("c2"))

            # AllGather the 8 partials (copy-only, ~2x faster than ncfw
            # AllReduce at this size), then tree-sum locally in fp32 on DVE
            cc_in = dram.tile([128, 16], DT)
            cc_out = dram.tile([NCORES * 128, 16], DT, addr_space="Shared")
            nc.sync.dma_start(cc_in[:], y2sb[:])
            nc.gpsimd.collective_compute(
                "AllGather", mybir.AluOpType.bypass,
                replica_groups=[list(range(NCORES))],
                ins=[cc_in[:].opt()], outs=[cc_out[:].opt()])
            y2all = sb.tile([128, NCORES * 16], DT)
            nc.sync.dma_start(
                y2all[:].rearrange("p (r f) -> p r f", r=NCORES),
                cc_out[:].rearrange("(r p) f -> p r f", p=128))
            ts1 = sb.tile([128, 64], DT)
            nc.vector.tensor_add(ts1[:], y2all[:, 0:64], y2all[:, 64:128])
            ts2 = sb.tile([128, 32], DT)
            nc.vector.tensor_add(ts2[:], ts1[:, 0:32], ts1[:, 32:64])
            y2f = sb.tile([128, 16], DT)
            nc.vector.tensor_add(y2f[:], ts2[:, 0:16], ts2[:, 16:32])

            u_sb = sb.tile([128, 16], BT)
            nc.scalar.activation(u_sb[:], y2f[:], AF.Relu, bias=zb[:])

            # ---- stage 4: fc_out (bin-sharded) + tanh ----
            # two PSUM banks so col0's tanh/products/osum overlap col1's
            # matmul stretch (single-bank serializes the read behind both)
            cpa = ps.tile([128, 1], DT, tag="b3")
            cpb = ps.tile([128, 1], DT, tag="b5")
            for j in range(16):
                s, jj = divmod(j, 8)
                base = WOC * jj
                nc.tensor.matmul(cpa[:],
                                 wot[s][:, base:base + 128],
                                 u_sb[:, j:j + 1], start=(j == 0), stop=(j == 15))
            for j in range(16):
                s, jj = divmod(j, 8)
                base = WOC * jj
                nc.tensor.matmul(cpb[0:82, :],
                                 wot[s][:, base + 128:base + WOC],
                                 u_sb[:, j:j + 1], start=(j == 0), stop=(j == 15))
            coef = sb.tile([128, 2], DT)
            nc.vector.memset(coef[:], 0.0)
            nc.scalar.activation(coef[:, 0:1], cpa[:], AF.Tanh,
                                 bias=smv("bout")[:, 0:1])
            p1 = sb.tile([128, 2], DT)
            p2 = sb.tile([128, 2], DT)
            nc.vector.tensor_mul(p1[:, 0:1], smv("x2")[:, 0:1], coef[:, 0:1])
            nc.vector.tensor_mul(p2[:, 0:1], smv("xs2")[:, 0:1], coef[:, 0:1])
            nc.scalar.activation(coef[0:82, 1:2], cpb[0:82, :], AF.Tanh,
                                 bias=smv("bout")[0:82, 1:2])
            nc.vector.tensor_mul(p1[:, 1:2], smv("x2")[:, 1:2], coef[:, 1:2])
            nc.vector.tensor_mul(p2[:, 1:2], smv("xs2")[:, 1:2], coef[:, 1:2])

            # ---- final: complex multiply + sum over clc_order ----
            osum = ps.tile([2 * FB, 1], DT, tag="b4")
            mre = smv("mre")
            mim = smv("mim")
            nc.tensor.matmul(osum[:], mre[:, 0:42], p1[:, 0:1],
                             start=True, stop=False)
            nc.tensor.matmul(osum[:], mim[:, 0:42], p2[:, 0:1],
                             start=False, stop=False)
            nc.tensor.matmul(osum[:], mre[:, 42:84], p1[:, 1:2],
                             start=False, stop=False)
            nc.tensor.matmul(osum[:], mim[:, 42:84], p2[:, 1:2],
                             start=False, stop=True)
            ot = sb.tile([2 * FB, 1], DT)
            nc.scalar.activation(ot[:], osum[:], AF.Copy)
            nc.sync.dma_start(o_out[:], ot[:])

    nc.compile()
    return nc


def prep_inputs(x, h_norm, h_rnn, W1, b1, g1, beta1, rm1, rv1,
                W_ih, W_hh, b_ih, b_hh, W2, b2, g2, beta2, rm2, rv2,
                Wout, bout):
    """Host-side prep: fold BN, transpose/pad/shard weights, pack smalls."""
    x = np.asarray(x, f32)
    h = np.asarray(h_rnn, f32).reshape(H)
    h_norm = np.asarray(h_norm, f32)

    # row5 (same on all cores)
    L4 = np.array([(1.0 - ALPHA) * ALPHA ** (4 - t) for t in range(5)], f32)
    row5 = np.zeros((5, R5_COLS), f32)
    row5[:, :IN] = x.reshape(5, IN)
    row5[0, IN:IN + F] = h_norm
    row5[:, IN + F] = L4
    row5[0, IN + F + 1] = ALPHA ** 5

    # fc1 + bn1 folded
    s1 = (np.asarray(g1, f32) / np.sqrt(np.asarray(rv1, f32) + BN_EPS))
    W1s = np.asarray(W1, f32) * s1[:, None]
    c1 = (np.asarray(b1, f32) - np.asarray(rm1, f32)) * s1 + np.asarray(beta1, f32)
    W1sT = np.zeros((K1, K1), f32)
    W1sT[:IN, :IN] = W1s.T
    w1t_img = _img(W1sT, 3, K1)
    c1_pf = _pf(np.pad(c1, (0, K1 - IN)), 3)

    x4d50 = np.pad(x[O - 1].reshape(IN) / 50.0, (0, K1 - IN)).astype(f32)
    x4_pf = _pf(x4d50, 3)

    # fc2 + bn2 folded
    s2 = (np.asarray(g2, f32) / np.sqrt(np.asarray(rv2, f32) + BN_EPS))
    W2s = np.asarray(W2, f32) * s2[:, None]
    c2 = (np.asarray(b2, f32) - np.asarray(rm2, f32)) * s2 + np.asarray(beta2, f32)
    c2_pf = _pf(c2 / NCORES, 16)
    hpf = _pf(h, 16)

    bsum = np.asarray(b_ih, f32) + np.asarray(b_hh, f32)
    W_ih = np.asarray(W_ih, f32)
    W_hh = np.asarray(W_hh, f32)
    Wout = np.asarray(Wout, f32)
    bout = np.asarray(bout, f32)
    wob = Wout.reshape(O, F, 2, H)
    bob = bout.reshape(O, F, 2)

    # +-1 reduction matrices for the final complex-mul (same on all cores)
    Mre = np.zeros((256, 2 * FB), f32)
    Mim = np.zeros((256, 2 * FB), f32)
    for t in range(WOC):
        o_, gci = divmod(t, 2 * FB)
        g, ci = divmod(gci, 2)
        Mre[t, 2 * g] = 1.0 if ci == 0 else -1.0
        Mim[t, 2 * g + 1] = 1.0
    mre_img = _img(Mre, 2, 2 * FB)
    mim_img = _img(Mim, 2, 2 * FB)

    in_maps = []
    for c in range(NCORES):
        gr = np.arange(PCH * c, PCH * (c + 1))
        idx = np.concatenate([gr, H + gr, 2 * H + gr])
        WihT = np.zeros((K1, GR), f32)
        WihT[:IN, :] = W_ih[idx, :].T
        wih_img = _img(WihT, 3, GR).astype(bf16)
        whh_img = _img(np.ascontiguousarray(W_hh[idx, :].T), 16, GR).astype(bf16)

        brz_pf = _pf(np.concatenate([bsum[gr], bsum[H + gr]]), 4)
        bin_pf = _pf(np.asarray(b_ih, f32)[2 * H + gr], 2)
        bhn_pf = _pf(np.asarray(b_hh, f32)[2 * H + gr], 2)
        hown_pf = _pf(h[gr], 2)

        w2_img = _img(np.ascontiguousarray(W2s[:, gr].T), 2, H).astype(bf16)

        # Wout rows for this core's bins, o-major
        WoT = np.zeros((H, WOC), f32)
        bo_c = np.zeros(WOC, f32)
        x2v = np.zeros(256, f32)
        xs2v = np.zeros(256, f32)
        f0 = FB * c
        for o_ in range(O):
            for g in range(FB):
                f = f0 + g
                if f >= F:
                    continue
                for ci in range(2):
                    t = o_ * 2 * FB + 2 * g + ci
                    WoT[:, t] = wob[o_, f, ci, :]
                    bo_c[t] = bob[o_, f, ci]
                    x2v[t] = 2.0 * x[o_, f, ci]
                    xs2v[t] = 2.0 * x[o_, f, 1 - ci]
        wot_img = _img(WoT, 16, WOC).astype(bf16)
        bout_pf = _pf(np.pad(bo_c, (0, 256 - WOC)), 2)

        smalls = np.zeros((128, SM_COLS), f32)

        def put(name, arr):
            o_, w = _SM_LAYOUT[name]
            smalls[:, o_:o_ + w] = arr

        put("x4", x4_pf)
        put("c1", c1_pf)
        put("brz", brz_pf)
        put("bin", bin_pf)
        put("bhn", bhn_pf)
        put("hpf", hpf)
        put("hown", hown_pf)
        put("c2", c2_pf)
        put("bout", bout_pf)
        put("x2", _pf(x2v, 2))
        put("xs2", _pf(xs2v, 2))
        put("mre", mre_img)
        put("mim", mim_img)

        in_maps.append({
            "row5": row5, "smalls": smalls, "w1t": w1t_img,
            "wih": wih_img, "whh": whh_img, "w2t": w2_img, "wot": wot_img,
            "h16": hpf.astype(bf16),
        })
    return in_maps


def assemble(results):
    out_full = np.concatenate(
        [results[c]["o_out"].ravel() for c in range(NCORES)])[:IN]
    out = out_full.reshape(F, 2).astype(f32)
    h_norm_new = results[0]["o_hn"].reshape(F).astype(f32)
    h_new = np.concatenate(
        [results[c]["o_hnew"].T.ravel() for c in range(NCORES)])
    return out, h_norm_new, h_new.reshape(1, 1, H).astype(f32)


_NC_CACHE = [None]


def run(inputs, trace=False, tmpdir=None):
    if _NC_CACHE[0] is None:
        _NC_CACHE[0] = build_nc()
    nc = _NC_CACHE[0]
    in_maps = prep_inputs(**inputs)
    res = run_bass_kernel_spmd(nc, in_maps, core_ids=list(range(NCORES)),
                               trace=trace, tmpdir=tmpdir)
    return assemble(res.results), res


def kernel(**inputs):
    (out, h_norm_new, h_new), _ = run(inputs)
    return out, h_norm_new, h_new
